# revision 51
# baseline (speedup 1.0000x reference)
"""CRF-RNN layer (nn_CRF_RNN_Layer) as a Bass/Tile kernel on 8 trn2 NeuronCores.

Math (reference):
    N = 96*96 pixels, C = 21 classes, 5 mean-field iterations.
    k_spatial / k_bilateral are [N, N] Gaussian kernels; per iteration:
        p = softmax(q); S = Ks @ p; Bi = Kb @ p
        pairwise = (S * ws + Bi * wb) @ C.T;  q = u - pairwise

Device strategy:
    - Row-shard outputs over 8 cores (BAND = 1152 rows each); channels
      padded 21 -> 32 with -1e30 logits so softmax pads are 0.
    - Both Gaussian kernels decay fast (theta_gamma=3, theta_alpha=8), so
      the j-contraction is truncated to a banded per-band-half window
      around each half's rows: spatial halo 4 j-tiles (of 128),
      bilateral halo ~9 (validated rel err ~3.6e-3 vs 2e-2 budget).
      Windows live in a per-core ROTATED frame: the SBUF p-window holds
      3 ranks (r-1..r+1, own band at slots [9,18)) so the SPMD program
      is static; per-core variation is input data plus one runtime rank
      offset (snapped once from partition_id) used by dynamic-offset
      DMAs.
    - Ks: host constant, fp8, streamed to SBUF.  Kb: built on device as
      exp(-S/128) where S = 64||dFc||^2 + dy^2 + dx^2 comes from one
      9-feature fp16 matmul (positions recentered per core; integer
      arithmetic exact), scalar-engine exp writes fp8 directly.
    - Iterations: out.T-form fp8 DoubleRow matmuls accumulate
      S.T/Bi.T [32, band] in PSUM; compat+weights fold on host into
      csf/cbf so the pairwise step is a pair of [CP,CP] matmuls that
      also transpose back to pixel-major.  Both halves' accumulations
      are interleaved in one dependency-class-sorted stream (own band
      first, then each half-fill as it lands) so that after the last
      fill only class-3 pairs and the tails remain — this hides most of
      the AllGather + re-gather latency.
    - p0 = softmax(u) computed on host (kills the first AllGather).
      Each later iteration softmaxes per band-half, copies its own
      band straight into the next p-window, and stages both halves
      into ONE AllGather per iteration (two AGs serialize on the CC
      engine); the gathered fp8 buffer is ring-extended in DRAM and
      the two neighbor ranks re-gathered with single dynamic-offset
      DMAs on the sync queue.
    - A tiny AllGather fires first-thing to absorb NEFF launch skew
      under the Kb build.
"""

import numpy as np
import ml_dtypes

from concourse import bacc, mybir, tile
from concourse.ap import AP
from concourse.bass_utils import run_bass_kernel_spmd

H, W, C = 96, 96, 21
THETA_ALPHA, THETA_BETA, THETA_GAMMA = 8.0, 0.125, 3.0
NITER = 5
NCORES = 8
N = H * W                     # 9216
BAND = N // NCORES            # 1152 rows per core
CP = 32                       # padded channels
TB = BAND // 128              # 9 band tiles
TJ = N // 128                 # 72 j-tiles total
NEG = -1.0e30

# window geometry (slots are 128-pixel tiles in the rotated frame where
# the core's own band occupies absolute slots [36, 45) of [0, 72)).
NF = 9                        # fp16 features for the Kb exponent
NRANK_W = 3                   # p-window spans ranks r-1..r+1
NW = NRANK_W * TB             # 27 slots in the SBUF p-window
SP_LO, HS_T = 5, 18           # spatial window: p-win slots [5, 23)
BI_LO, HB_T = 0, 27           # bilateral window: p-win slots [0, 27)
RANKSEG = 128 * TB * CP       # elements per rank in the ring-ext buffer

HALVES = [  # (band col offset, col len, tile offset, n tiles, psum chunks)
    (0, 512, 0, 4, [(0, 512)]),
    (512, 640, 4, 5, [(0, 512), (512, 128)]),
]

_CACHE = {}
USE_KEEPERS = False


def _slot_class(slot):
    """Availability class of a p-window slot within an iteration:
    0 = own band, A half (local copy at A-tail, pre-AllGather)
    1 = own band, B half
    2/3 = remote (lands with the fills after the iteration's single
    AllGather; kept distinct only for issue-order heuristics)"""
    K, t = slot // TB, slot % TB
    if K == 1:
        return 0 if t < 4 else 1
    return 2 if t < 4 else 3


def _pairs(lo, ntiles, reorder):
    """DR pairs (s, s+1) of p-window slots; when reorder, order by how
    early the pair's inputs land so AllGather latency hides under
    already-runnable matmuls."""
    ps = [lo + 2 * k for k in range(ntiles // 2)]
    if not reorder:
        return ps
    return sorted(ps, key=lambda s: max(_slot_class(s), _slot_class(s + 1)))


# per-half j-windows: each band half only contracts over j within its own
# halo reach (same minimum per-row halo as the validated shared window)
HALF_WIN = [  # half -> (spatial lo, n, bilateral lo, n)
    (5, 12, 0, 22),     # half A: band slots [9,13) -> sp [5,17) bi [0,22)
    (9, 14, 3, 24),     # half B: band slots [13,18) -> sp [9,23) bi [3,27)
]


def _build_nc():
    nc = bacc.Bacc("TRN2", target_bir_lowering=False, debug=False,
                   num_devices=NCORES)
    f32 = mybir.dt.float32
    f16 = mybir.dt.float16
    fp8 = mybir.dt.float8e4

    uband_d = nc.declare_dram_parameter("uband", [128, TB * CP], f32, isOutput=False)
    p0_d = nc.declare_dram_parameter("p0", [128, NW * CP], fp8, isOutput=False)
    kst_d = nc.declare_dram_parameter("kst", [128, HS_T * BAND], fp8, isOutput=False)
    atw_d = nc.declare_dram_parameter("atw", [NF, HB_T * 128], f16, isOutput=False)
    btw_d = nc.declare_dram_parameter("btw", [NF, BAND], f16, isOutput=False)
    csf_d = nc.declare_dram_parameter("csf", [CP, CP], f32, isOutput=False)
    cbf_d = nc.declare_dram_parameter("cbf", [CP, CP], f32, isOutput=False)
    out_d = nc.declare_dram_parameter("out", [128, TB * CP], f32, isOutput=True)

    with tile.TileContext(nc) as tc:
        with (
            tc.tile_pool(name="kres", bufs=1) as kres,
            tc.tile_pool(name="state", bufs=1) as state,
            tc.tile_pool(name="small", bufs=1) as small,
            tc.tile_pool(name="pwin", bufs=2) as pwin_pool,
            tc.tile_pool(name="dram", bufs=1, space="DRAM") as dram,
            tc.tile_pool(name="accsA", bufs=1, space="PSUM") as accsA_pool,
            tc.tile_pool(name="accbA", bufs=1, space="PSUM") as accbA_pool,
            tc.tile_pool(name="pwp", bufs=1, space="PSUM") as pw_pool,
            tc.tile_pool(name="xp2", bufs=1, space="PSUM") as xp2_pool,
        ):
            # build-phase psum pool, closed before the B-half acc pools
            # open so both halves' accumulators fit the 8 PSUM banks
            _xp_cm = tc.tile_pool(name="xp", bufs=3, space="PSUM")
            xp_pool = _xp_cm.__enter__()

            # one runtime scalar: base rank of the 3-rank window = pid-1
            pid = nc.sync.partition_id()
            off = nc.sync.snap((pid + NCORES - 1) % NCORES, min_val=0,
                               max_val=NCORES - 1)

            # skew-absorbing dummy barrier, triggered first
            zb = small.tile([1, 4], f32, tag="zb")
            nc.vector.memset(zb[:], 0.0)
            bar_in = dram.tile([4], f32, tag="barin")
            bar_out = dram.tile([4 * NCORES], f32, addr_space="Shared", tag="barout")
            nc.gpsimd.dma_start(bar_in.rearrange("(p f) -> p f", p=1)[:], zb[:])
            nc.gpsimd.collective_compute(
                "AllGather", mybir.AluOpType.bypass,
                ins=[bar_in[:]], outs=[bar_out[:]],
                replica_groups=[list(range(NCORES))],
            )

            # constants — build deps (atw/btw) first so the build starts
            # immediately; Ks streams on the sync queue in parallel
            csf = state.tile([CP, CP], f32, tag="csf")
            cbf = state.tile([CP, CP], f32, tag="cbf")
            u_band = state.tile([128, TB * CP], f32, tag="uband")
            atw = state.tile([NF, HB_T * 128], f16, tag="atw")
            btw = state.tile([NF, BAND], f16, tag="btw")
            nc.scalar.dma_start(atw[:], atw_d[:])
            nc.scalar.dma_start(btw[:], btw_d[:])
            nc.scalar.dma_start(csf[:], csf_d[:])
            nc.scalar.dma_start(cbf[:], cbf_d[:])
            nc.scalar.dma_start(u_band[:], uband_d[:])

            pwin0 = pwin_pool.tile([128, NW * CP], fp8, tag="pwin")
            nc.sync.dma_start(pwin0[:], p0_d[:])

            ks_res = kres.tile([128, HS_T * BAND], fp8, tag="ksres")
            KSG = 6  # slots per Ks streaming DMA
            for w in range(0, HS_T, KSG):
                wl = min(KSG, HS_T - w)
                nc.sync.dma_start(
                    ks_res[:, w * BAND:(w + wl) * BAND],
                    kst_d[:, w * BAND:(w + wl) * BAND],
                )

            # ---- build Kb = exp(-S/128), S from 9-feature f16 matmul ----
            kb_res = kres.tile([128, HB_T * BAND], fp8, tag="kbres")
            BCH = [(0, 512), (512, 512), (1024, 128)]

            def emit_build(v, pool):
                lhs = atw[:, v * 128:(v + 1) * 128]
                for co, cl in BCH:
                    xp = pool.tile([128, 512], f32, tag="xp", name="xp")
                    nc.tensor.matmul(xp[:, :cl], lhs, btw[:, co:co + cl],
                                     start=True, stop=True)
                    nc.scalar.activation(
                        kb_res[:, v * BAND + co: v * BAND + co + cl],
                        xp[:, :cl], mybir.ActivationFunctionType.Exp,
                        scale=-1.0 / 128.0,
                    )

            # only the slots iteration-0's A half consumes are built up
            # front; the rest is emitted after it0's A-tail so the first
            # AllGather triggers earlier
            BUILD_SPLIT = HB_T   # deferring slots past it0-A measured no
            # better than building everything up front (1-buf psum
            # serialization offsets the earlier AllGather trigger)
            for v in range(BUILD_SPLIT):
                emit_build(v, xp_pool)

            ks3 = ks_res.rearrange("p (s i) -> p s i", s=HS_T)
            kb3 = kb_res.rearrange("p (s i) -> p s i", s=HB_T)

            # free the build psum banks, then open the B-half acc pools
            _xp_cm.__exit__(None, None, None)
            _accB_cm = tc.tile_pool(name="accsB", bufs=1, space="PSUM")
            accsB_pool = _accB_cm.__enter__()
            _accbB_cm = tc.tile_pool(name="accbB", bufs=1, space="PSUM")
            accbB_pool = _accbB_cm.__enter__()
            acc_pools = [(accsA_pool, accbA_pool), (accsB_pool, accbB_pool)]

            # ---- iterations (both halves' accumulations interleaved in
            # one class-sorted stream: after the B-fill lands only the
            # class-3 pairs of each half plus the tails remain) ----
            pwin_cur = pwin0
            for it in range(NITER):
                last = it == NITER - 1
                pw3 = pwin_cur.rearrange("p (s c) -> p s c", c=CP)
                if not last:
                    pwin_next = pwin_pool.tile([128, NW * CP], fp8, tag="pwin")
                    pn4 = pwin_next.rearrange("p (K t c) -> p K t c",
                                              K=NRANK_W, t=TB)
                    ext = dram.tile([10 * RANKSEG], fp8, tag=f"ext{it}")
                    ext4 = ext.rearrange("(r p t c) -> r p t c", r=10, p=128,
                                         t=TB)
                    ag_in = dram.tile([RANKSEG], fp8, tag=f"agin{it}")
                    ag_out = dram.tile([NCORES * RANKSEG], fp8,
                                       addr_space="Shared", tag=f"agout{it}")
                accs = []
                for hi, (coff, clen, toff, nt, chunks) in enumerate(HALVES):
                    sp, bp = acc_pools[hi]
                    accs.append((
                        [sp.tile([CP, cl], f32, tag=f"accs{hi}{ci}",
                                 name=f"accs{hi}{ci}")
                         for ci, (co, cl) in enumerate(chunks)],
                        [bp.tile([CP, cl], f32, tag=f"accb{hi}{ci}",
                                 name=f"accb{hi}{ci}")
                         for ci, (co, cl) in enumerate(chunks)],
                    ))

                def emit_tail(hi):
                    coff, clen, toff, nt, chunks = HALVES[hi]
                    acc_s, acc_b = accs[hi]
                    st = small.tile([CP, 640], f32, tag=f"st{hi}")
                    bit = small.tile([CP, 640], f32, tag=f"bit{hi}")
                    for ci, (co, cl) in enumerate(chunks):
                        nc.scalar.copy(st[:, co:co + cl], acc_s[ci][:, :cl])
                        nc.vector.tensor_copy(bit[:, co:co + cl],
                                              acc_b[ci][:, :cl])
                    pw = pw_pool.tile([128, 5 * CP], f32, tag="pw")
                    for ic in range(nt):
                        nc.tensor.matmul(
                            pw[:, ic * CP:(ic + 1) * CP],
                            st[:, ic * 128:(ic + 1) * 128], csf[:],
                            start=True, stop=False,
                        )
                        nc.tensor.matmul(
                            pw[:, ic * CP:(ic + 1) * CP],
                            bit[:, ic * 128:(ic + 1) * 128], cbf[:],
                            start=False, stop=True,
                        )
                    qnew = small.tile([128, 5 * CP], f32, tag=f"qnew{toff}")
                    nc.vector.tensor_tensor(
                        qnew[:, :nt * CP],
                        u_band[:, toff * CP:(toff + nt) * CP],
                        pw[:, :nt * CP], op=mybir.AluOpType.subtract,
                    )
                    if last:
                        nc.sync.dma_start(
                            out_d[:, toff * CP:(toff + nt) * CP],
                            qnew[:, :nt * CP],
                        )
                        return
                    eb = small.tile([128, 5 * CP], f32, tag=f"eb{toff}")
                    sums = small.tile([128, 5], f32, tag=f"sums{toff}")
                    for t in range(nt):
                        nc.scalar.activation(
                            eb[:, t * CP:(t + 1) * CP],
                            qnew[:, t * CP:(t + 1) * CP],
                            mybir.ActivationFunctionType.Exp,
                            accum_out=sums[:, t:t + 1],
                        )
                    rb = small.tile([128, 5], f32, tag=f"rb{toff}")
                    nc.vector.reciprocal(rb[:, :nt], sums[:, :nt])
                    pband = small.tile([128, 5 * CP], fp8, tag=f"pband{toff}")
                    nc.vector.tensor_tensor(
                        pband.rearrange("p (t c) -> p t c", c=CP)[:, :nt, :],
                        eb.rearrange("p (t c) -> p t c", c=CP)[:, :nt, :],
                        rb[:, :nt].unsqueeze(2).to_broadcast((128, nt, CP)),
                        op=mybir.AluOpType.mult,
                    )
                    # own band straight into the next window (pre-AllGather)
                    nc.sync.dma_start(
                        pn4[:, 1, toff:toff + nt, :],
                        pband.rearrange("p (t c) -> p t c", c=CP)[:, :nt, :],
                    )
                    # stage this half into the shared per-iteration
                    # AllGather input (scalar HW queue — fast)
                    nc.scalar.dma_start(
                        ag_in.rearrange("(p t c) -> p t c", p=128,
                                        t=TB)[:, toff:toff + nt, :],
                        pband.rearrange("p (t c) -> p t c", c=CP)[:, :nt, :],
                    )

                stream = []
                for hi, (coff, clen, toff, nt, chunks) in enumerate(HALVES):
                    slo, snt, blo, bnt = HALF_WIN[hi]
                    for ker, lo, ntl, base in (
                        ("s", slo, snt, SP_LO), ("b", blo, bnt, BI_LO),
                    ):
                        for s in _pairs(lo, ntl, reorder=False):
                            klass = max(_slot_class(s), _slot_class(s + 1))
                            stream.append((klass, hi, ker, s, base))
                if it > 0:
                    # class order (remote classes are one fill event
                    # now); half A's remote pairs first so its tail and
                    # staging fire earliest
                    stream.sort(key=lambda x: (min(x[0], 2),
                                               x[1] if x[0] >= 2 else 0))
                firsts, lasts = {}, {}
                for i, (_, hi, ker, _, _) in enumerate(stream):
                    firsts.setdefault((hi, ker), i)
                    lasts[(hi, ker)] = i
                last_of_half = {hi: max(i for i, e in enumerate(stream)
                                        if e[1] == hi) for hi in (0, 1)}
                for i, (klass, hi, ker, s, lo) in enumerate(stream):
                    coff, clen, toff, nt, chunks = HALVES[hi]
                    acc = accs[hi][0] if ker == "s" else accs[hi][1]
                    K3 = ks3 if ker == "s" else kb3
                    first = i == firsts[(hi, ker)]
                    lastp = i == lasts[(hi, ker)]
                    for ci, (co, cl) in enumerate(chunks):
                        nc.tensor.matmul(
                            acc[ci][:, :cl],
                            pw3[:, s:s + 2, :],
                            K3[:, s - lo:s - lo + 2,
                               coff + co:coff + co + cl],
                            start=first, stop=lastp,
                            perf_mode=mybir.MatmulPerfMode.DoubleRow,
                        )
                    if i == last_of_half[0]:
                        emit_tail(0)
                        if it == 0:
                            # rest of the Kb build (only it0's B half and
                            # later iterations need these slots)
                            for v in range(BUILD_SPLIT, HB_T):
                                emit_build(v, xp2_pool)
                emit_tail(1)
                if not last:
                    # single AllGather per iteration (two serialized on
                    # the CC engine cost ~6us extra), then ring-extend
                    # and two full-segment neighbor fills
                    nc.gpsimd.collective_compute(
                        "AllGather", mybir.AluOpType.bypass,
                        ins=[ag_in[:]], outs=[ag_out[:]],
                        replica_groups=[list(range(NCORES))],
                    )
                    agv = ag_out.rearrange("(r p t c) -> r p t c", r=NCORES,
                                           p=128, t=TB)
                    nc.sync.dma_start(ext4[0:NCORES], agv[:])
                    nc.sync.dma_start(ext4[NCORES:10], agv[0:2])
                    extv = ext.rearrange("(r p t c) -> p r t c", r=10, p=128,
                                         t=TB)
                    dynoff = off * RANKSEG
                    for klo in (0, 2):
                        srcf = extv[:, klo:klo + 1, :, :]
                        dynf = AP(srcf.tensor, srcf.offset + dynoff, srcf.ap)
                        nc.sync.dma_start(pn4[:, klo:klo + 1, :, :], dynf)
                    pwin_cur = pwin_next

            _accbB_cm.__exit__(None, None, None)
            _accB_cm.__exit__(None, None, None)

    nc.compile()
    return nc


def _host_inputs(unaries, reference_image, spatial_ker_weights,
                 bilateral_ker_weights, compatibility_matrix):
    u = np.asarray(unaries, np.float32).reshape(N, C)
    img = np.asarray(reference_image, np.float32).reshape(N, 3)
    ws = np.asarray(spatial_ker_weights, np.float32)
    wb = np.asarray(bilateral_ker_weights, np.float32)
    comp = np.asarray(compatibility_matrix, np.float32)

    yy, xx = np.meshgrid(np.arange(H, dtype=np.float64),
                         np.arange(W, dtype=np.float64), indexing="ij")
    Y, X = yy.ravel(), xx.ravel()

    # padded u (pixel-major band tiles) and folded compat
    u_pad = np.full((N, CP), NEG, np.float32)
    u_pad[:, :C] = u
    csf = np.zeros((CP, CP), np.float32)
    cbf = np.zeros((CP, CP), np.float32)
    csf[:C, :C] = ws[:, None] * comp.T
    cbf[:C, :C] = wb[:, None] * comp.T

    # p0 = softmax(u), fp8, padded
    e = np.exp(u - u.max(1, keepdims=True))
    p0 = np.zeros((N, CP), np.float32)
    p0[:, :C] = e / e.sum(1, keepdims=True)
    p0 = p0.astype(ml_dtypes.float8_e4m3)

    Fc = ((img - 0.5) / THETA_BETA).astype(np.float64)
    sq64 = 64.0 * (Fc * Fc).sum(1)

    in_maps = []
    for r in range(NCORES):
        band = np.arange(r * BAND, (r + 1) * BAND)
        cy = 12.0 * r + 6.0

        def abs_tile(slot):   # rotated slot -> absolute j-tile
            return (9 * (r - 1) + slot) % TJ

        # Ks window, fp8, [128, w, i] layout
        kst = np.empty((128, HS_T, BAND), ml_dtypes.float8_e4m3)
        yi, xi = Y[band], X[band]
        for w in range(HS_T):
            t = abs_tile(SP_LO + w)
            j = np.arange(t * 128, (t + 1) * 128)
            d2 = (Y[j, None] - yi[None, :]) ** 2 + (X[j, None] - xi[None, :]) ** 2
            kst[:, w, :] = np.exp(
                d2 * (-0.5 / (THETA_GAMMA * THETA_GAMMA))
            ).astype(ml_dtypes.float8_e4m3)

        # bilateral features (A for window j, B for band i), f16
        jidx = np.concatenate(
            [np.arange(abs_tile(BI_LO + v) * 128, abs_tile(BI_LO + v) * 128 + 128)
             for v in range(HB_T)]
        )
        yj, xj = Y[jidx] - cy, X[jidx] - 48.0
        yi_c, xi_c = yi - cy, xi - 48.0
        atw = np.stack([
            sq64[jidx] + yj * yj,
            -128.0 * Fc[jidx, 0], -128.0 * Fc[jidx, 1], -128.0 * Fc[jidx, 2],
            np.ones(HB_T * 128), np.ones(HB_T * 128),
            -2.0 * yj, xj * xj, -2.0 * xj,
        ], 0).astype(np.float16)
        btw = np.stack([
            np.ones(BAND),
            Fc[band, 0], Fc[band, 1], Fc[band, 2],
            sq64[band] + yi_c * yi_c, xi_c * xi_c,
            yi_c, np.ones(BAND), xi_c,
        ], 0).astype(np.float16)

        # p0 window: slot 9K+t -> absolute rank (r+K-2)%8, tile t
        p0w = np.empty((128, NW, CP), ml_dtypes.float8_e4m3)
        for Krank in range(NRANK_W):
            ar = (r + Krank - 1) % NCORES
            blk = p0[ar * BAND:(ar + 1) * BAND].reshape(TB, 128, CP)
            p0w[:, Krank * TB:(Krank + 1) * TB, :] = blk.transpose(1, 0, 2)

        uband = (
            u_pad[band].reshape(TB, 128, CP).transpose(1, 0, 2)
            .reshape(128, TB * CP)
        )
        in_maps.append({
            "uband": uband,
            "p0": p0w.reshape(128, NW * CP),
            "kst": kst.reshape(128, HS_T * BAND),
            "atw": atw,
            "btw": btw,
            "csf": csf,
            "cbf": cbf,
        })
    return in_maps


def _run(in_maps, trace=False, **kw):
    if "nc" not in _CACHE:
        _CACHE["nc"] = _build_nc()
    return run_bass_kernel_spmd(
        _CACHE["nc"], in_maps, list(range(NCORES)), trace=trace, **kw
    )


def _assemble(results):
    bands = []
    for r in range(NCORES):
        arr = results[r]["out"]                              # [128, TB*CP]
        band = arr.reshape(128, TB, CP).transpose(1, 0, 2).reshape(BAND, CP)
        bands.append(band[:, :C])
    return np.concatenate(bands, axis=0).reshape(1, H, W, C).astype(np.float32)


def kernel(unaries, reference_image, spatial_ker_weights,
           bilateral_ker_weights, compatibility_matrix):
    in_maps = _host_inputs(
        unaries, reference_image, spatial_ker_weights,
        bilateral_ker_weights, compatibility_matrix,
    )
    res = _run(in_maps, trace=False)
    return _assemble(res.results)
